# revision 14
# baseline (speedup 1.0000x reference)
"""AllusionBERT-CRF loss kernel for 8 TRN2 NeuronCores.

Data-parallel over packed tokens: each sequence only needs its first
need_b = max(mask_len_b, spans_end_b) tokens on device (emissions past
the mask are discarded by the CRF recursion; attention scores are only
read below the span ends). The host packs those prefixes, LPT-balances
them across the 8 cores, transposes each core's shard to [768, NT] and
casts to fp8e4 (~2.3 MB/core at the reference length distribution vs
3.1 MB unpacked). Weights (att_W1|W_pos fused, x32) are fp8e4 too so
the device can run DoubleRow matmuls: 3 chunk-pair matmuls per
512-token block at 2 MACs/cell/cycle, half the PE streaming time of
the fp16-stationary version. Input chunk DMAs are split across the SP
and ACT hardware DGE queues (L halves first) so compute starts as
early as possible; drains alternate DVE/ACT into a [67, NT] fp8
staging tile that is written back per column half. The host unpacks
z|em, rescales, and runs the tiny sequential CRF recursion, span
softmax and focal loss exactly.
"""

import os
import sys

import numpy as np

for _p in ("/opt/trn_rl_repo",):
    if _p not in sys.path and os.path.isdir(_p):
        sys.path.insert(0, _p)

B, S, H, T, M = 64, 512, 768, 500, 8
N_CORES = 8
P = 128
KC = H // P                  # 6 contraction chunks (3 DoubleRow pairs)
NOUT = 64 + 3                # fused cols: 64 att hidden first, then 3 emissions
MP = 80                      # stationary col pad: fp8 k-plane step must be %16 B
BLK = 512                    # PSUM bank columns
WSCALE = 32.0                # weight scale into fp8e4 normal range
POSITION_WEIGHT = 0.6
LABEL_SMOOTH = 0.1
GAMMA = 2.0

_STATE = {}


def _install_ntff_hook():
    """Register the axon NTFF profile hook that the container's antenv stub
    lacks, so run_bass_kernel_spmd(trace=True) can report exec_time_ns."""
    import contextlib
    import ctypes
    import types

    if "antenv.axon_hooks" in sys.modules:
        return
    try:
        lib = ctypes.CDLL("/opt/axon/libaxon_pjrt.so")
        if not hasattr(lib, "axon_start_nrt_profile"):
            return
    except OSError:
        return
    lib.axon_start_nrt_profile.argtypes = [
        ctypes.POINTER(ctypes.c_int64),
        ctypes.c_size_t,
    ]
    lib.axon_start_nrt_profile.restype = ctypes.c_int64
    lib.axon_stop_nrt_profile.argtypes = [ctypes.c_char_p]
    lib.axon_stop_nrt_profile.restype = ctypes.c_int64

    @contextlib.contextmanager
    def _hook(output_dir, device_ids):
        import jax

        jax.devices()
        if device_ids:
            ids = (ctypes.c_int64 * len(device_ids))(*device_ids)
            rc = lib.axon_start_nrt_profile(ids, len(device_ids))
        else:
            rc = lib.axon_start_nrt_profile(None, 0)
        if rc != 0:
            raise RuntimeError(f"axon_start_nrt_profile rc={rc}")
        try:
            yield
        finally:
            n = lib.axon_stop_nrt_profile(str(output_dir).encode())
            print(f"ntff profile: {n} file(s) written to {output_dir}")

    mod = types.ModuleType("antenv.axon_hooks")
    _hooks = {"ntff": _hook}
    mod.get_axon_ntff_profile_hook = lambda: _hooks["ntff"]

    def _set(h):
        _hooks["ntff"] = h

    mod.set_axon_ntff_profile_hook = _set
    sys.modules["antenv.axon_hooks"] = mod


def _build(NT):
    import concourse.bacc as bacc
    import concourse.bass as bass
    import concourse.mybir as mybir
    import concourse.tile as tile

    f32 = mybir.dt.float32
    f8 = mybir.dt.float8e4
    DR = mybir.MatmulPerfMode.DoubleRow
    nc = bacc.Bacc(None, target_bir_lowering=False)

    HALF = NT // 2
    NBLK = NT // BLK
    ht = nc.declare_dram_parameter("ht", [H, NT], f8, isOutput=False)
    wc = nc.declare_dram_parameter("wc", [P, KC * MP], f8, isOutput=False)
    # output padded to 128 partitions, block-major so each per-block DMA is
    # one fully-contiguous DRAM write: anything less defeats the HWDGE's
    # 16-engine split and the transfer crawls on a single engine.
    zem = nc.declare_dram_parameter("zem", [NBLK, P, BLK], f8, isOutput=True)

    NWU = 5  # HAM warmup matmuls: fill DMA wait time with PE activity

    with tile.TileContext(nc) as tc:
        with (
            tc.tile_pool(name="sbuf", bufs=1) as cpool,
            tc.tile_pool(
                name="psum", bufs=min(NBLK + 1, 8), space=bass.MemorySpace.PSUM
            ) as pp,
        ):
            wu_sb = cpool.tile([P, BLK], f8)
            wc_sb = cpool.tile([P, KC, MP], f8)
            h_sb = cpool.tile([P, KC, NT], f8)
            zem_sb = cpool.tile([P, NT], f8)

            # gpsimd: warmup memset, then the small weights load on the
            # software-DGE queue where it is alone (on a shared HWDGE queue
            # its 480 B packets round-robin against the big hidden chunks
            # and it doesn't finish until the stream is nearly done, gating
            # the first ldweights), then the staging pad memset (rows 67:128
            # stay junk; the host ignores them but the DMA reads them).
            nc.gpsimd.memset(wu_sb[:], 0.0)
            nc.gpsimd.dma_start(wc_sb[:], wc[:])
            nc.gpsimd.memset(zem_sb[64:P, :], 0.0)

            wu_po = pp.tile([P, BLK], f32, tag="po")
            for _ in range(NWU):
                nc.tensor.matmul(
                    wu_po[:], wu_sb[:, 0:P], wu_sb[:], start=True, stop=True
                )

            # chunk halves alternate between the SP and ACT hardware DGE
            # queues so pairs arrive together and the left half lands
            # before the right.
            for half in range(2):
                c0, c1 = half * HALF, (half + 1) * HALF
                for k in range(KC):
                    eng = nc.sync if k % 2 == 0 else nc.scalar
                    eng.dma_start(
                        h_sb[:, k, c0:c1], ht[k * P:(k + 1) * P, c0:c1]
                    )

            po = {}
            for half in range(2):
                blks = range(half * (NBLK // 2), (half + 1) * (NBLK // 2))
                for kp in range(KC // 2):
                    for b in blks:
                        if kp == 0:
                            po[b] = pp.tile(
                                [NOUT, BLK], f32, name=f"po{b}", tag="po"
                            )
                        nc.tensor.matmul(
                            po[b][:],
                            wc_sb[:, 2 * kp:2 * kp + 2, 0:NOUT],
                            h_sb[:, 2 * kp:2 * kp + 2, b * BLK:(b + 1) * BLK],
                            start=(kp == 0),
                            stop=(kp == KC // 2 - 1),
                            perf_mode=DR,
                        )
                # drains alternate ACT/DVE (gpsimd cannot read PSUM), each
                # followed by its own contiguous 64 KB block DMA so only
                # the final block's write is tail-serial
                for i, b in enumerate(blks):
                    dst = zem_sb[0:NOUT, b * BLK:(b + 1) * BLK]
                    if i % 2 == 0:
                        nc.scalar.activation(
                            dst, po[b][:],
                            mybir.ActivationFunctionType.Identity,
                        )
                    else:
                        nc.vector.tensor_copy(dst, po[b][:])
                    nc.sync.dma_start(
                        zem[b], zem_sb[:, b * BLK:(b + 1) * BLK]
                    )

    nc.compile()
    return nc


def _pack_plan(attention_mask, target_positions):
    """Per-sequence needed token counts, LPT-balanced onto the 8 cores."""
    am = np.asarray(attention_mask)
    tp = np.asarray(target_positions).astype(np.int64)
    lengths = (am != 0).sum(1).astype(np.int64)
    ends = tp[..., 1].max(1)
    need = np.clip(np.maximum(lengths, ends), 1, S)
    order = np.argsort(-need, kind="stable")
    bins = [[] for _ in range(N_CORES)]
    loads = np.zeros(N_CORES, dtype=np.int64)
    for b in order:
        j = int(loads.argmin())
        bins[j].append(int(b))
        loads[j] += need[b]
    NT = max(1024, int(-(-loads.max() // 1024)) * 1024)
    return need, bins, NT


def _run_device(hidden, W_pos, att_W1, need, bins, NT):
    import ml_dtypes
    from concourse.bass_utils import run_bass_kernel_spmd

    key = ("nc", NT)
    if key not in _STATE:
        _STATE[key] = _build(NT)
    nc = _STATE[key]

    f8 = ml_dtypes.float8_e4m3

    # fused weights [768, 67] -> [128, 6, 80] fp8, scaled into normal range
    wcat = np.concatenate([att_W1, W_pos], axis=1) * WSCALE
    wcv = np.zeros((P, KC, MP), f8)
    wcv[:, :, 0:NOUT] = wcat.reshape(KC, P, NOUT).transpose(1, 0, 2).astype(f8)
    wcv = wcv.reshape(P, KC * MP)

    h8 = hidden.astype(f8)  # [B, S, H]
    in_maps = []
    for core in range(N_CORES):
        htc = np.zeros((H, NT), f8)
        off = 0
        for b in bins[core]:
            n = int(need[b])
            htc[:, off:off + n] = h8[b, :n].T
            off += n
        in_maps.append({"ht": htc, "wc": wcv})

    trace = os.environ.get("KERNEL_TRACE", "0") == "1"
    if trace:
        _install_ntff_hook()
    try:
        res = run_bass_kernel_spmd(
            nc, in_maps, core_ids=list(range(N_CORES)), trace=trace
        )
    except Exception:
        if not trace:
            raise
        res = run_bass_kernel_spmd(nc, in_maps, core_ids=list(range(N_CORES)))
    _STATE["exec_time_ns"] = getattr(res, "exec_time_ns", None)

    z = np.zeros((B, S, 64), np.float32)
    em = np.zeros((B, S, 3), np.float32)
    inv = 1.0 / WSCALE
    for core in range(N_CORES):
        zh = np.asarray(res.results[core]["zem"])  # [NBLK, 128, BLK]
        zc = zh[:, 0:NOUT].transpose(1, 0, 2).reshape(NOUT, NT).astype(
            np.float32
        )  # [67, NT]
        off = 0
        for b in bins[core]:
            n = int(need[b])
            blkv = zc[:, off:off + n].T * inv  # [n, 67]
            z[b, :n] = blkv[:, 0:64]
            em[b, :n] = blkv[:, 64:67]
            off += n
    return z, em


def _logsumexp(x, axis):
    m = np.max(x, axis=axis, keepdims=True)
    return np.squeeze(m, axis) + np.log(np.sum(np.exp(x - m), axis=axis))


def kernel(hidden, attention_mask, position_labels, type_labels, target_positions,
           bi_label_weight, W_pos, b_pos, start_trans, end_trans, trans,
           att_W1, att_b1, att_W2, att_b2, W_type, b_type):
    hidden = np.asarray(hidden, dtype=np.float32)
    need, bins, NT = _pack_plan(attention_mask, target_positions)
    z, em_raw = _run_device(
        hidden,
        np.asarray(W_pos, np.float32),
        np.asarray(att_W1, np.float32),
        need, bins, NT,
    )
    emissions = em_raw.astype(np.float64) + np.asarray(b_pos, np.float64)
    zb = z + np.asarray(att_b1, np.float32)
    scores = (
        np.tanh(zb) @ np.asarray(att_W2, np.float32)
    )[..., 0].astype(np.float64) + float(np.asarray(att_b2).reshape(-1)[0])

    mask = np.asarray(attention_mask).astype(bool)
    labels = np.asarray(position_labels).astype(np.int64)
    trans = np.asarray(trans, np.float64)
    start_trans = np.asarray(start_trans, np.float64)
    end_trans = np.asarray(end_trans, np.float64)
    blw = float(np.asarray(bi_label_weight))

    w = np.where(labels > 0, 1.0 + blw, 1.0)[..., None]
    em = emissions * w

    # --- CRF NLL ---
    maskf = mask.astype(np.float64)
    emit = np.take_along_axis(em, labels[..., None], -1)[..., 0]
    emit_score = (emit * maskf).sum(1)
    tr = trans[labels[:, :-1], labels[:, 1:]]
    tr_score = (tr * maskf[:, 1:]).sum(1)
    last = maskf.sum(1).astype(np.int64) - 1
    last_tags = np.take_along_axis(labels, last[:, None], 1)[:, 0]
    score = start_trans[labels[:, 0]] + emit_score + tr_score + end_trans[last_tags]

    alpha = start_trans[None, :] + em[:, 0]
    for t in range(1, S):
        nxt = _logsumexp(alpha[:, :, None] + trans[None, :, :] + em[:, t][:, None, :], 1)
        alpha = np.where(mask[:, t][:, None], nxt, alpha)
    logZ = _logsumexp(alpha + end_trans[None, :], -1)
    position_loss = (logZ - score).mean()

    # --- span attention pooling + focal type loss ---
    tp = np.asarray(target_positions).astype(np.int64)
    starts, ends = tp[..., 0], tp[..., 1]
    valid = tp.sum(-1) > 0
    # spans only cover tokens < ends.max(); restrict the pooling window
    smax = int(ends.max()) if ends.size else S
    smax = max(min(smax, S), 1)
    pos = np.arange(smax)
    span_mask = (pos[None, None, :] >= starts[..., None]) & (pos[None, None, :] < ends[..., None])
    att = np.where(span_mask, scores[:, None, :smax], -1e9)
    att = att - att.max(-1, keepdims=True)
    aw = np.exp(att)
    aw = aw / aw.sum(-1, keepdims=True)
    pooled = np.einsum('bms,bsh->bmh', aw, hidden[:, :smax].astype(np.float64))
    logits = pooled @ np.asarray(W_type, np.float64) + np.asarray(b_type, np.float64)

    tl = np.asarray(type_labels).astype(np.int64)
    onehot = np.eye(T)[tl]
    smooth = onehot * (1.0 - LABEL_SMOOTH) + LABEL_SMOOTH / T
    lz = logits - logits.max(-1, keepdims=True)
    logp = lz - np.log(np.exp(lz).sum(-1, keepdims=True))
    probs = np.exp(logp)
    ce = -(smooth * logp).sum(-1)
    pt = (smooth * probs).sum(-1)
    focal = ce * (1.0 - pt) ** GAMMA
    v = valid.astype(np.float64)
    type_loss = (focal * v).sum() / max(v.sum(), 1.0) * 10.0

    joint = POSITION_WEIGHT * position_loss + (1.0 - POSITION_WEIGHT) * type_loss
    return np.array([joint, position_loss, type_loss], dtype=np.float32)


# revision 18
# speedup vs baseline: 1.0527x; 1.0527x over previous
"""AllusionBERT-CRF loss kernel for 8 TRN2 NeuronCores.

Data-parallel over packed tokens: each sequence only needs its first
need_b = max(mask_len_b, spans_end_b) tokens on device (emissions past
the mask are discarded by the CRF recursion; attention scores are only
read below the span ends). The host packs those prefixes, LPT-balances
them across the 8 cores, transposes each core's shard to [768, NT] and
casts to fp8e4 (~2.3 MB/core at the reference length distribution vs
3.1 MB unpacked). Weights (att_W1|W_pos fused, x32) are fp8e4 too so
the device can run DoubleRow matmuls: 3 chunk-pair matmuls per
512-token block at 2 MACs/cell/cycle, half the PE streaming time of
the fp16-stationary version. Input chunk DMAs are split across the SP
and ACT hardware DGE queues (L halves first) so compute starts as
early as possible; drains alternate DVE/ACT into a [67, NT] fp8
staging tile that is written back per column half. The host unpacks
z|em, rescales, and runs the tiny sequential CRF recursion, span
softmax and focal loss exactly.
"""

import os
import sys

import numpy as np

for _p in ("/opt/trn_rl_repo",):
    if _p not in sys.path and os.path.isdir(_p):
        sys.path.insert(0, _p)

B, S, H, T, M = 64, 512, 768, 500, 8
N_CORES = 8
P = 128
KC = H // P                  # 6 contraction chunks (3 DoubleRow pairs)
NOUT = 64 + 3                # fused cols: 64 att hidden first, then 3 emissions
MP = 80                      # stationary col pad: fp8 k-plane step must be %16 B
BLK = 512                    # PSUM bank columns
WSCALE = 32.0                # weight scale into fp8e4 normal range
POSITION_WEIGHT = 0.6
LABEL_SMOOTH = 0.1
GAMMA = 2.0

_STATE = {}


def _install_ntff_hook():
    """Register the axon NTFF profile hook that the container's antenv stub
    lacks, so run_bass_kernel_spmd(trace=True) can report exec_time_ns."""
    import contextlib
    import ctypes
    import types

    if "antenv.axon_hooks" in sys.modules:
        return
    try:
        lib = ctypes.CDLL("/opt/axon/libaxon_pjrt.so")
        if not hasattr(lib, "axon_start_nrt_profile"):
            return
    except OSError:
        return
    lib.axon_start_nrt_profile.argtypes = [
        ctypes.POINTER(ctypes.c_int64),
        ctypes.c_size_t,
    ]
    lib.axon_start_nrt_profile.restype = ctypes.c_int64
    lib.axon_stop_nrt_profile.argtypes = [ctypes.c_char_p]
    lib.axon_stop_nrt_profile.restype = ctypes.c_int64

    @contextlib.contextmanager
    def _hook(output_dir, device_ids):
        import jax

        jax.devices()
        if device_ids:
            ids = (ctypes.c_int64 * len(device_ids))(*device_ids)
            rc = lib.axon_start_nrt_profile(ids, len(device_ids))
        else:
            rc = lib.axon_start_nrt_profile(None, 0)
        if rc != 0:
            raise RuntimeError(f"axon_start_nrt_profile rc={rc}")
        try:
            yield
        finally:
            n = lib.axon_stop_nrt_profile(str(output_dir).encode())
            print(f"ntff profile: {n} file(s) written to {output_dir}")

    mod = types.ModuleType("antenv.axon_hooks")
    _hooks = {"ntff": _hook}
    mod.get_axon_ntff_profile_hook = lambda: _hooks["ntff"]

    def _set(h):
        _hooks["ntff"] = h

    mod.set_axon_ntff_profile_hook = _set
    sys.modules["antenv.axon_hooks"] = mod


def _build(NT):
    import concourse.bacc as bacc
    import concourse.bass as bass
    import concourse.mybir as mybir
    import concourse.tile as tile

    f32 = mybir.dt.float32
    f8 = mybir.dt.float8e4
    DR = mybir.MatmulPerfMode.DoubleRow
    nc = bacc.Bacc(None, target_bir_lowering=False)

    HALF = NT // 2
    NBLK = NT // BLK
    ht = nc.declare_dram_parameter("ht", [H, NT], f8, isOutput=False)
    wc = nc.declare_dram_parameter("wc", [P, KC * MP], f8, isOutput=False)
    # output padded to 128 partitions, one fully-contiguous DRAM write per
    # column half: anything less defeats the HWDGE's 16-engine split and
    # the transfer crawls on a single engine.
    zem = nc.declare_dram_parameter("zem", [2, P, HALF], f8, isOutput=True)

    NWU = 8  # HAM warmup matmuls: fill DMA wait time with PE activity

    with tile.TileContext(nc) as tc:
        with (
            tc.tile_pool(name="sbuf", bufs=1) as cpool,
            tc.tile_pool(
                name="psum", bufs=min(NBLK + 1, 8), space=bass.MemorySpace.PSUM
            ) as pp,
        ):
            wu_sb = cpool.tile([P, BLK], f8)
            wc_sb = cpool.tile([P, KC, MP], f8)
            h_sb = cpool.tile([P, KC, NT], f8)
            zs = [cpool.tile([P, HALF], f8, name=f"zs{h}") for h in range(2)]

            # DVE is idle until the first drain: give it the warmup memset
            # so the HAM warmup matmuls start as early as possible.
            nc.vector.memset(wu_sb[:], 0.0)
            # gpsimd: the small weights load goes on the software-DGE queue
            # where it is alone (on a shared HWDGE queue its 480 B packets
            # round-robin against the big hidden chunks and it doesn't
            # finish until the stream is nearly done, gating the first
            # ldweights), then the staging pad memsets (rows 67:128 stay
            # junk; the host ignores them but the DMAs read them).
            nc.gpsimd.dma_start(wc_sb[:], wc[:])
            for h in range(2):
                nc.gpsimd.memset(zs[h][64:P, :], 0.0)

            wu_po = pp.tile([P, BLK], f32, tag="po")
            for _ in range(NWU):
                nc.tensor.matmul(
                    wu_po[:], wu_sb[:, 0:P], wu_sb[:], start=True, stop=True
                )

            # chunk halves alternate between the SP and ACT hardware DGE
            # queues so pairs arrive together and the left half lands
            # before the right.
            for half in range(2):
                c0, c1 = half * HALF, (half + 1) * HALF
                for k in range(KC):
                    eng = nc.sync if k % 2 == 0 else nc.scalar
                    eng.dma_start(
                        h_sb[:, k, c0:c1], ht[k * P:(k + 1) * P, c0:c1]
                    )

            po = {}
            for half in range(2):
                blks = range(half * (NBLK // 2), (half + 1) * (NBLK // 2))
                for kp in range(KC // 2):
                    for b in blks:
                        if kp == 0:
                            po[b] = pp.tile(
                                [NOUT, BLK], f32, name=f"po{b}", tag="po"
                            )
                        nc.tensor.matmul(
                            po[b][:],
                            wc_sb[:, 2 * kp:2 * kp + 2, 0:NOUT],
                            h_sb[:, 2 * kp:2 * kp + 2, b * BLK:(b + 1) * BLK],
                            start=(kp == 0),
                            stop=(kp == KC // 2 - 1),
                            perf_mode=DR,
                        )
                # drains alternate ACT/DVE (gpsimd cannot read PSUM); the
                # half's out-DMA is issued by ACT for the left half (so it
                # overlaps the right-half stream without touching the sync
                # queue) and by sync for the right half (ACT still has
                # right-half drains to run)
                for i, b in enumerate(blks):
                    lb = (b - half * (NBLK // 2)) * BLK
                    dst = zs[half][0:NOUT, lb:lb + BLK]
                    if i % 2 == 0:
                        nc.scalar.activation(
                            dst, po[b][:],
                            mybir.ActivationFunctionType.Identity,
                        )
                    else:
                        nc.vector.tensor_copy(dst, po[b][:])
                out_eng = nc.scalar if half == 0 else nc.sync
                out_eng.dma_start(zem[half], zs[half][:])

    nc.compile()
    return nc


def _pack_plan(attention_mask, target_positions):
    """Per-sequence needed token counts, LPT-balanced onto the 8 cores."""
    am = np.asarray(attention_mask)
    tp = np.asarray(target_positions).astype(np.int64)
    lengths = (am != 0).sum(1).astype(np.int64)
    ends = tp[..., 1].max(1)
    need = np.clip(np.maximum(lengths, ends), 1, S)
    order = np.argsort(-need, kind="stable")
    bins = [[] for _ in range(N_CORES)]
    loads = np.zeros(N_CORES, dtype=np.int64)
    for b in order:
        j = int(loads.argmin())
        bins[j].append(int(b))
        loads[j] += need[b]
    NT = max(1024, int(-(-loads.max() // 1024)) * 1024)
    return need, bins, NT


def _run_device(hidden, W_pos, att_W1, need, bins, NT):
    import ml_dtypes
    from concourse.bass_utils import run_bass_kernel_spmd

    key = ("nc", NT)
    if key not in _STATE:
        _STATE[key] = _build(NT)
    nc = _STATE[key]

    f8 = ml_dtypes.float8_e4m3

    # fused weights [768, 67] -> [128, 6, 80] fp8, scaled into normal range
    wcat = np.concatenate([att_W1, W_pos], axis=1) * WSCALE
    wcv = np.zeros((P, KC, MP), f8)
    wcv[:, :, 0:NOUT] = wcat.reshape(KC, P, NOUT).transpose(1, 0, 2).astype(f8)
    wcv = wcv.reshape(P, KC * MP)

    h8 = hidden.astype(f8)  # [B, S, H]
    in_maps = []
    for core in range(N_CORES):
        htc = np.zeros((H, NT), f8)
        off = 0
        for b in bins[core]:
            n = int(need[b])
            htc[:, off:off + n] = h8[b, :n].T
            off += n
        in_maps.append({"ht": htc, "wc": wcv})

    trace = os.environ.get("KERNEL_TRACE", "0") == "1"
    if trace:
        _install_ntff_hook()
    try:
        res = run_bass_kernel_spmd(
            nc, in_maps, core_ids=list(range(N_CORES)), trace=trace
        )
    except Exception:
        if not trace:
            raise
        res = run_bass_kernel_spmd(nc, in_maps, core_ids=list(range(N_CORES)))
    _STATE["exec_time_ns"] = getattr(res, "exec_time_ns", None)

    z = np.zeros((B, S, 64), np.float32)
    em = np.zeros((B, S, 3), np.float32)
    inv = 1.0 / WSCALE
    for core in range(N_CORES):
        zh = np.asarray(res.results[core]["zem"])  # [2, 128, NT/2]
        zc = np.concatenate(
            [zh[0, 0:NOUT], zh[1, 0:NOUT]], axis=1
        ).astype(np.float32)  # [67, NT]
        off = 0
        for b in bins[core]:
            n = int(need[b])
            blkv = zc[:, off:off + n].T * inv  # [n, 67]
            z[b, :n] = blkv[:, 0:64]
            em[b, :n] = blkv[:, 64:67]
            off += n
    return z, em


def _logsumexp(x, axis):
    m = np.max(x, axis=axis, keepdims=True)
    return np.squeeze(m, axis) + np.log(np.sum(np.exp(x - m), axis=axis))


def kernel(hidden, attention_mask, position_labels, type_labels, target_positions,
           bi_label_weight, W_pos, b_pos, start_trans, end_trans, trans,
           att_W1, att_b1, att_W2, att_b2, W_type, b_type):
    hidden = np.asarray(hidden, dtype=np.float32)
    need, bins, NT = _pack_plan(attention_mask, target_positions)
    z, em_raw = _run_device(
        hidden,
        np.asarray(W_pos, np.float32),
        np.asarray(att_W1, np.float32),
        need, bins, NT,
    )
    emissions = em_raw.astype(np.float64) + np.asarray(b_pos, np.float64)
    zb = z + np.asarray(att_b1, np.float32)
    scores = (
        np.tanh(zb) @ np.asarray(att_W2, np.float32)
    )[..., 0].astype(np.float64) + float(np.asarray(att_b2).reshape(-1)[0])

    mask = np.asarray(attention_mask).astype(bool)
    labels = np.asarray(position_labels).astype(np.int64)
    trans = np.asarray(trans, np.float64)
    start_trans = np.asarray(start_trans, np.float64)
    end_trans = np.asarray(end_trans, np.float64)
    blw = float(np.asarray(bi_label_weight))

    w = np.where(labels > 0, 1.0 + blw, 1.0)[..., None]
    em = emissions * w

    # --- CRF NLL ---
    maskf = mask.astype(np.float64)
    emit = np.take_along_axis(em, labels[..., None], -1)[..., 0]
    emit_score = (emit * maskf).sum(1)
    tr = trans[labels[:, :-1], labels[:, 1:]]
    tr_score = (tr * maskf[:, 1:]).sum(1)
    last = maskf.sum(1).astype(np.int64) - 1
    last_tags = np.take_along_axis(labels, last[:, None], 1)[:, 0]
    score = start_trans[labels[:, 0]] + emit_score + tr_score + end_trans[last_tags]

    alpha = start_trans[None, :] + em[:, 0]
    for t in range(1, S):
        nxt = _logsumexp(alpha[:, :, None] + trans[None, :, :] + em[:, t][:, None, :], 1)
        alpha = np.where(mask[:, t][:, None], nxt, alpha)
    logZ = _logsumexp(alpha + end_trans[None, :], -1)
    position_loss = (logZ - score).mean()

    # --- span attention pooling + focal type loss ---
    tp = np.asarray(target_positions).astype(np.int64)
    starts, ends = tp[..., 0], tp[..., 1]
    valid = tp.sum(-1) > 0
    # spans only cover tokens < ends.max(); restrict the pooling window
    smax = int(ends.max()) if ends.size else S
    smax = max(min(smax, S), 1)
    pos = np.arange(smax)
    span_mask = (pos[None, None, :] >= starts[..., None]) & (pos[None, None, :] < ends[..., None])
    att = np.where(span_mask, scores[:, None, :smax], -1e9)
    att = att - att.max(-1, keepdims=True)
    aw = np.exp(att)
    aw = aw / aw.sum(-1, keepdims=True)
    pooled = np.einsum('bms,bsh->bmh', aw, hidden[:, :smax].astype(np.float64))
    logits = pooled @ np.asarray(W_type, np.float64) + np.asarray(b_type, np.float64)

    tl = np.asarray(type_labels).astype(np.int64)
    onehot = np.eye(T)[tl]
    smooth = onehot * (1.0 - LABEL_SMOOTH) + LABEL_SMOOTH / T
    lz = logits - logits.max(-1, keepdims=True)
    logp = lz - np.log(np.exp(lz).sum(-1, keepdims=True))
    probs = np.exp(logp)
    ce = -(smooth * logp).sum(-1)
    pt = (smooth * probs).sum(-1)
    focal = ce * (1.0 - pt) ** GAMMA
    v = valid.astype(np.float64)
    type_loss = (focal * v).sum() / max(v.sum(), 1.0) * 10.0

    joint = POSITION_WEIGHT * position_loss + (1.0 - POSITION_WEIGHT) * type_loss
    return np.array([joint, position_loss, type_loss], dtype=np.float32)
